# revision 4
# baseline (speedup 1.0000x reference)
"""FFPS sampler (furthest-point sampling on concatenated point+feature
distances) for Trainium2, B=4 x N=4096 x C=128, npoint=1024.

Strategy: one NeuronCore per batch element (cores 4-7 run redundant copies).
Per core:
  phase 1: dist = aa_i + aa_j - 2 * f @ f^T computed on the tensor engine
           (fp32, K split 128+3, aa_j folded in as a rank-1 row of the
           second matmul, aa_i added in the PSUM->SBUF epilogue), streamed
           to an internal HBM buffer (64 MB).
  phase 2: 1024 sequential FPS steps. Each step gathers one dist row from
           HBM with a register-indexed DMA, updates the running min-distance
           in SBUF, and computes the argmax via per-partition max/max_index
           plus a PE-transpose cross-partition reduction.
"""
import sys
sys.path.insert(0, "/opt/trn_rl_repo")
import numpy as np

_B, _N, _C, _NP = 4, 4096, 128, 1024
_D = 3 + _C          # 131
_NCOLS = _N // 128   # 32
_MB, _NB = 128, 512  # dist block sizes
_NM, _NN = _N // _MB, _N // _NB
_UNROLL = 16

_state = {}


def _split_multi_waits(nc, max_waits=1):
    """This walrus build rejects instructions carrying >1 semaphore waits;
    move extras onto dedicated no-ops placed just before."""
    import concourse.mybir as mb
    for fn in nc.m.functions:
        for blk in fn.blocks:
            new_insts = []
            for inst in blk.instructions:
                si = inst.sync_info
                if si and si.on_wait and len(si.on_wait) > max_waits:
                    extras = list(si.on_wait[max_waits:])
                    del si.on_wait[max_waits:]
                    for k, w in enumerate(extras):
                        nop = mb.InstNoOp(name=f"{inst.name}-ws{k}",
                                          ins=[], outs=[])
                        nop.engine = inst.engine
                        nop.sync_info = mb.SyncInfo(on_wait=[w], on_update=[])
                        new_insts.append(nop)
                new_insts.append(inst)
            blk.instructions[:] = new_insts


def build_program():
    import concourse.bass as bass
    import concourse.tile as tile
    import concourse.mybir as mybir

    f32 = mybir.dt.float32
    i32 = mybir.dt.int32
    u32 = mybir.dt.uint32
    Alu = mybir.AluOpType
    AF = mybir.ActivationFunctionType

    nc = bass.Bass("TRN2", target_bir_lowering=False)
    fT0D = nc.dram_tensor("fT0", [128, _N], f32, kind="ExternalInput")
    L1D = nc.dram_tensor("L1", [4, _N], f32, kind="ExternalInput")
    fT1D = nc.dram_tensor("fT1", [3, _N], f32, kind="ExternalInput")
    faD = nc.dram_tensor("fa", [128, _NCOLS * _D], f32, kind="ExternalInput")
    outD = nc.dram_tensor("idx", [1, _NP], i32, kind="ExternalOutput")
    distD = nc.dram_tensor("dist", [_N, _N], f32, kind="Internal")
    dist_v = distD.ap().rearrange("a (b c) -> a b c", b=128)

    with tile.TileContext(nc) as tc:
        with tc.tile_pool(name="big", bufs=1) as bigp, \
             tc.tile_pool(name="stage", bufs=2) as stp, \
             tc.tile_pool(name="ps", bufs=4, space="PSUM") as psp, \
             tc.tile_pool(name="psT", bufs=2, space="PSUM") as psTp, \
             tc.tile_pool(name="fps", bufs=1) as pool, \
             tc.tile_pool(name="fpsps", bufs=1, space="PSUM") as pps:

            # ---------------- phase 1: distance matrix ----------------
            fT0 = bigp.tile([128, _N], f32)
            L1 = bigp.tile([4, _N], f32)
            R1 = bigp.tile([4, _N], f32)
            fa = bigp.tile([128, _NCOLS * _D], f32)
            aa_pm = bigp.tile([128, _NCOLS], f32)
            aarow = bigp.tile([1, _N], f32)
            ident = bigp.tile([128, 128], f32)
            ones = bigp.tile([128, 128], f32)

            nc.gpsimd.dma_start(fT0[:], fT0D.ap())
            nc.gpsimd.dma_start(L1[:], L1D.ap())
            nc.gpsimd.dma_start(R1[0:3, :], fT1D.ap())
            nc.gpsimd.dma_start(fa[:], faD.ap())

            nc.vector.memset(ones[:], 1.0)
            nc.gpsimd.affine_select(ident[:], ones[:], pattern=[[-1, 128]],
                                    compare_op=Alu.is_equal, fill=0.0,
                                    base=0, channel_multiplier=1)

            # aa_pm[p, m] = sum_k f[m*128+p, k]^2
            nc.vector.tensor_tensor(out=fa[:], in0=fa[:], in1=fa[:],
                                    op=Alu.mult)
            nc.vector.tensor_reduce(
                aa_pm[:], fa[:].rearrange("p (m k) -> p m k", k=_D),
                axis=mybir.AxisListType.X, op=Alu.add)

            # aarow[0, j] = -0.5*aa_j  (PE transposes of aa_pm columns)
            for m in range(_NM):
                psT = psTp.tile([1, 128], f32)
                nc.tensor.matmul(psT[:], aa_pm[:, m:m + 1], ident[:],
                                 start=True, stop=True)
                nc.scalar.activation(aarow[0:1, m * 128:(m + 1) * 128],
                                     psT[:], AF.Copy, scale=-0.5)
            nc.gpsimd.dma_start(R1[3:4, :], aarow[:])

            # dist blocks: psum = f@f^T - 0.5*aa_j ; out = -2*psum + aa_i
            for m in range(_NM):
                sbrow = stp.tile([128, _N], f32)
                for n in range(_NN):
                    ps = psp.tile([128, _NB], f32)
                    nc.tensor.matmul(ps[:], fT0[:, m * _MB:(m + 1) * _MB],
                                     fT0[:, n * _NB:(n + 1) * _NB],
                                     start=True, stop=False)
                    nc.tensor.matmul(ps[:], L1[:, m * _MB:(m + 1) * _MB],
                                     R1[:, n * _NB:(n + 1) * _NB],
                                     start=False, stop=True)
                    nc.vector.tensor_scalar(sbrow[:, n * _NB:(n + 1) * _NB],
                                            ps[:], scalar1=-2.0,
                                            scalar2=aa_pm[:, m:m + 1],
                                            op0=Alu.mult, op1=Alu.add)
                nc.gpsimd.dma_start(
                    distD.ap()[m * _MB:(m + 1) * _MB, :], sbrow[:])

            # ---------------- phase 2: sequential FPS ----------------
            mind = pool.tile([128, _NCOLS], f32)
            d = pool.tile([128, _NCOLS], f32)
            M8 = pool.tile([128, 8], f32)
            PJ = pool.tile([128, 1], f32)
            mi8 = pool.tile([128, 8], u32)
            TSm = pool.tile([1, 128], f32)
            TSj = pool.tile([1, 128], f32)
            gm8 = pool.tile([1, 8], f32)
            pen = pool.tile([1, 128], f32)
            cand = pool.tile([1, 128], f32)
            jmin = pool.tile([1, 1], f32)
            jint = pool.tile([1, 1], i32)
            ringA = pool.tile([1, _UNROLL], i32)
            ringB = pool.tile([1, _UNROLL], i32)
            jbase = pool.tile([128, 1], f32)
            jbi = pool.tile([128, 1], i32)
            TPa = pps.tile([8, 128], f32)
            TPb = pps.tile([1, 128], f32)

            nc.vector.memset(mind[:], 1e10)
            nc.gpsimd.iota(jbi[:], pattern=[[0, 1]], base=0,
                           channel_multiplier=_NCOLS)
            nc.vector.tensor_copy(jbase[:], jbi[:])
            nc.vector.memset(jint[:], 0)

            jreg = nc.sync.alloc_register("jreg")
            jval = nc.sync.snap(jreg, donate=True, min_val=0, max_val=_N - 1)
            blk_no = [0]

            def step(u, ring):
                nc.sync.reg_load(jreg, jint[0:1, 0:1])
                nc.sync.reg_save(ring[0:1, u:u + 1], jreg)
                nc.sync.dma_start(d[:], dist_v[bass.ds(jval, 1), :, :])
                nc.vector.tensor_tensor(out=mind[:], in0=mind[:], in1=d[:],
                                        op=Alu.min)
                nc.vector.max(M8[:], mind[:])
                nc.vector.max_index(mi8[:], M8[:], mind[:])
                nc.vector.tensor_add(PJ[:], jbase[:], mi8[:, 0:1])
                nc.tensor.transpose(TPa[:], M8[:], ident[:])
                nc.tensor.transpose(TPb[:], PJ[:], ident[:])
                nc.vector.tensor_copy(TSm[:], TPa[0:1, :])
                nc.vector.tensor_copy(TSj[:], TPb[:])
                nc.vector.max(gm8[:], TSm[:])
                nc.vector.tensor_scalar(pen[:], TSm[:],
                                        scalar1=gm8[0:1, 0:1], scalar2=1e9,
                                        op0=Alu.is_lt, op1=Alu.mult)
                nc.vector.tensor_add(cand[:], pen[:], TSj[:])
                nc.vector.tensor_reduce(jmin[:], cand[:],
                                        axis=mybir.AxisListType.X, op=Alu.min)
                nc.vector.tensor_copy(jint[:], jmin[:])

            def body(iv0, unroll):
                ring = ringA if blk_no[0] % 2 == 0 else ringB
                blk_no[0] += 1
                for u in range(unroll):
                    step(u, ring)
                nc.sync.dma_start(outD.ap()[0:1, bass.ds(iv0, unroll)],
                                  ring[0:1, 0:unroll])

            tc.For_i_unrolled_general(0, _NP, 1, body, max_unroll=_UNROLL)

    _split_multi_waits(nc)
    return nc


def _get_nc():
    if "nc" not in _state:
        _state["nc"] = build_program()
    return _state["nc"]


def prep_core_inputs(points, features, b):
    f = np.concatenate(
        [np.asarray(points[b], np.float32),
         np.ascontiguousarray(np.asarray(features[b], np.float32).T)],
        axis=1)                                   # [N, 131]
    fT = np.ascontiguousarray(f.T)                # [131, N]
    fa = np.ascontiguousarray(
        f.reshape(_N // 128, 128, _D).transpose(1, 0, 2).reshape(128, -1))
    L1 = np.concatenate([fT[128:], np.ones((1, _N), np.float32)], 0)
    return {"fT0": np.ascontiguousarray(fT[:128]), "L1": L1,
            "fT1": np.ascontiguousarray(fT[128:]), "fa": fa}


def kernel(points, features, npoint):
    from concourse.bass_utils import run_bass_kernel_spmd
    points = np.asarray(points)
    features = np.asarray(features)
    npoint = int(npoint)
    assert points.shape == (_B, _N, 3) and features.shape == (_B, _C, _N)
    assert npoint == _NP
    ins = [prep_core_inputs(points, features, c % _B) for c in range(8)]
    nc = _get_nc()
    last_err = None
    for _attempt in range(3):
        try:
            res = run_bass_kernel_spmd(nc, ins, core_ids=list(range(8)))
            break
        except Exception as e:  # transient NRT device errors: retry
            last_err = e
    else:
        raise last_err
    out = np.stack([res.results[b]["idx"][0] for b in range(_B)])
    return out.astype(np.int32)


if __name__ == "__main__":
    rng = np.random.default_rng(0)
    pts = rng.standard_normal((_B, _N, 3)).astype(np.float32)
    feats = rng.standard_normal((_B, _C, _N)).astype(np.float32)
    idx = kernel(pts, feats, _NP)
    print(idx.shape, idx.dtype)
    print(idx[:, :8])
